# revision 35
# baseline (speedup 1.0000x reference)
"""Causal multi-head attention (B=4, T=2048, C=1024, H=16, HD=64) on 8 TRN2
NeuronCores.

Sharding: 2D — batch (4) x head-group (2 groups of 8 heads). Each core handles
one batch's tokens for 8 heads:
    core = b * 2 + g
    xT  [C, T]  = x[b].T                       (bf16)
    wqT [C, OC] = (8*Wq[g*OC:(g+1)*OC, :]).T   (bf16; x8 folded into exp scale)
    wkT analogous (x8), wvT = Wv[g].T, woT = Wo[:, g].T (bf16)
    yT  [C, T]  partial f32: y[b] = sum_g yT_g.T  (host-side reduce over g)

On-chip layout strategy (no transposes anywhere):
  - Projections and the output matmul run fully in bf16 (weight loads get
    FWL, ~2.3x cheaper than f32r; inputs half the HBM traffic; PSUM f32).
    Weight DMAs ride the Activation HW DGE queue so their transfers overlap
    the x loads on the SP queue at startup.
  - Q^T per head / K^T per head-pair stored as fp8e4 [128, T]; Q zero-padded
    on the other head's channels so S contracts the full K=128 (FWL stays
    on — fp8 without DoubleRow runs at bf16 speed; DoubleRow measured
    SLOWER here, its weight loads cost +72%). Host pre-scales Wq/Wk by 8 so
    fp8 sees well-ranged values; the exp scale folds the 64x back out.
  - Causal trims: S matmuls and ctx matmuls skip the fully-masked column
    range of diagonal chunks (the base-0 diag pair keeps full-width S so
    one full-pair exp beats two sliced ones); exp runs per-chunk on the
    base -256 pair. Triangle masking multiplies P with one of two
    precomputed {0,1} bf16 mask tiles on the DVE.
  - V in [token, channel] bf16 with an ones column per head ([V_h | 1]) so
    the ctx matmul yields ctx^T rows 0..63 plus the softmax denominator in
    row 64 for free. ctx matmuls bf16 (fp8 V fails the accuracy budget).
  - Norm: copy sums row + ctx rows to SBUF (frees the PSUM bank fast),
    gpsimd partition_broadcast, DVE reciprocal_approx_fast (base-partition-0
    SBUF only — PSUM or offset-64 inputs silently misbehave on HW), DVE
    multiply into bf16 ct tiles.
  - y^T = woT_chunk.T @ ct; the last block's output is split ci 0-2 / ci 3
    so most of it overlaps the final attention head.
"""

import numpy as np

B, T_FULL, C = 4, 2048, 1024
H, HD = 16, 64
GROUPS = 2
HL = H // GROUPS          # heads per core = 8
OC = HL * HD              # local channels = 512
P = 128                   # partitions
TB = 512                  # token block (moving dim)
SCALE = float(1.0 / np.sqrt(HD))
WS = 8.0                  # host-side Wq/Wk scale (folded out in exp)
NCORES = 8


def build_program(T=T_FULL):
    import os
    from contextlib import ExitStack

    import concourse.bacc as bacc
    import concourse.mybir as mybir
    import concourse.tile as tile

    S_DR = os.environ.get("K_S_MODE", "plain") == "dr"
    NORM_OLD = os.environ.get("K_NORM", "new") == "old"
    MASK_GPS = os.environ.get("K_MASK", "dve") == "gpsimd"
    DEBUG_DUMP = os.environ.get("K_DEBUG", "") == "1"

    f32 = mybir.dt.float32
    bf16 = mybir.dt.bfloat16
    fp8 = mybir.dt.float8e4
    EXP = mybir.ActivationFunctionType.Exp
    GE = mybir.AluOpType.is_ge
    DR = mybir.MatmulPerfMode.DoubleRow

    NTB = T // TB             # 512-token blocks
    NKC = T // P              # 128-token key chunks
    CCH = C // P              # 8 contraction chunks of C
    MCH = OC // P             # 4 output-channel chunks

    nc = bacc.Bacc("TRN2", target_bir_lowering=False, debug=False)
    xT = nc.dram_tensor("xT", [C, T], bf16, kind="ExternalInput").ap()
    wqT = nc.dram_tensor("wqT", [C, OC], bf16, kind="ExternalInput").ap()
    wkT = nc.dram_tensor("wkT", [C, OC], bf16, kind="ExternalInput").ap()
    wvT = nc.dram_tensor("wvT", [C, OC], bf16, kind="ExternalInput").ap()
    woT = nc.dram_tensor("woT", [OC, C], bf16, kind="ExternalInput").ap()
    yT = nc.dram_tensor("yT", [C, T], f32, kind="ExternalOutput").ap()
    dbg = {}
    if DEBUG_DUMP:
        for nm, shape, dt_ in (
            ("d_qt0", [P, T], mybir.dt.float8e4),
            ("d_kt0", [P, T], mybir.dt.float8e4),
            ("d_v0", [P, HL * P], bf16), ("d_ct0", [P, T], bf16),
            ("d_msk0", [P, 2, TB], bf16), ("d_msk1", [P, 2, TB], bf16),
            ("d_wq", [P, C // P, OC], bf16), ("d_x", [P, C // P, TB], bf16),
        ):
            dbg[nm] = nc.dram_tensor(nm, shape, dt_, kind="ExternalOutput").ap()

    with tile.TileContext(nc) as tc, ExitStack() as ctx:
        perm = ctx.enter_context(tc.tile_pool(name="perm", bufs=1))
        # per-head Q tiles (fp8), zero-padded on the other head's 64 channels
        # so the S matmul can contract the full K=128 partitions (FWL path);
        # K^T tiles hold both heads of a pair.
        qt = [perm.tile([P, T], fp8, tag=f"qt{h}", name=f"qt{h}")
              for h in range(HL)]
        kt = [perm.tile([P, T], fp8, tag=f"kt{m}", name=f"kt{m}")
              for m in range(MCH)]
        ct = [perm.tile([P, T], bf16, tag=f"ct{m}", name=f"ct{m}")
              for m in range(MCH)]
        # V padded to 128 cols per head: [V_h | 1 | 0...] so ctx lhsT is M=128
        v = [perm.tile([P, HL * P], bf16, tag=f"v{t}", name=f"v{t}")
             for t in range(NKC)]
        # causal mask tiles for the two diagonal-pair alignments
        masks = [perm.tile([P, 2, TB], bf16, tag=f"msk{i}", name=f"msk{i}")
                 for i in range(2)]
        ONE_BF16 = 0x3F80  # 1.0 in bf16 — bf16 memset via uint16 bitcast
        for h in range(HL):
            z0 = (1 - h % 2) * 64  # zero rows: the other head's half
            nc.gpsimd.memset(qt[h][z0:z0 + 64, :].bitcast(mybir.dt.uint8), 0)
        for vt in v:
            vv = vt.rearrange("p (h e) -> p h e", e=P)
            nc.gpsimd.memset(vv[:, :, 64:65].bitcast(mybir.dt.uint16), ONE_BF16)
            nc.gpsimd.memset(vv[:, :, 65:].bitcast(mybir.dt.uint16), 0)
        for i, base in enumerate((0, -2 * P)):
            nc.gpsimd.memset(masks[i].bitcast(mybir.dt.uint16), ONE_BF16)
            nc.gpsimd.affine_select(
                out=masks[i], in_=masks[i], compare_op=GE, fill=0.0,
                base=base, pattern=[[-P, 2], [1, TB]], channel_multiplier=-1)

        # ---- Fused pipeline: project(tb) -> output(tb-1) -> attend(tb) ----
        # All pools coexist; PSUM budget (8 banks): mm512 2 + st 2x2 + ctx 2.
        with (
            tc.tile_pool(name="wpool", bufs=1) as wp,
            tc.tile_pool(name="xpool", bufs=2) as xp,
            tc.tile_pool(name="ptpool", bufs=4) as ptp,
            tc.tile_pool(name="tmppool", bufs=2) as tmp,
            tc.tile_pool(name="ypool", bufs=2) as yp,
            tc.tile_pool(name="mmps", bufs=2, space="PSUM") as pp,
            tc.tile_pool(name="stps", bufs=2, space="PSUM") as stp,
            tc.tile_pool(name="ctxps", bufs=2, space="PSUM") as cxp,
        ):
            HC = CCH // 2

            def load_x(tb):
                halves = []
                for s in range(2):
                    t_ = xp.tile([P, HC, TB], bf16, tag=f"x{s}",
                                 name=f"x_{tb}_{s}")
                    nc.sync.dma_start(
                        out=t_,
                        in_=xT.rearrange("(c p) t -> p c t", p=P)
                            [:, s * HC:(s + 1) * HC,
                             tb * TB:(tb + 1) * TB])
                    halves.append(t_)
                return halves

            def xsl(xc, c):
                return xc[c // HC][:, c % HC, :]

            x_next = load_x(0)
            # per-m weight tiles for Q/K so the first projection group's
            # weights land early (DMA issue for big 3-d patterns is ~1-3us)
            wq = [wp.tile([P, CCH, P], bf16, tag=f"wq{m}", name=f"wq{m}")
                  for m in range(MCH)]
            wk = [wp.tile([P, CCH, P], bf16, tag=f"wk{m}", name=f"wk{m}")
                  for m in range(MCH)]
            # weights go on the Activation HW DGE queue so their transfers run
            # in parallel with the x loads on the SP queue at startup
            for m in range(MCH):
                nc.scalar.dma_start(
                    out=wq[m],
                    in_=wqT[:, m * P:(m + 1) * P]
                        .rearrange("(c p) o -> p c o", p=P))
            for m in range(MCH):
                nc.scalar.dma_start(
                    out=wk[m],
                    in_=wkT[:, m * P:(m + 1) * P]
                        .rearrange("(c p) o -> p c o", p=P))
            wv = wp.tile([P, CCH, OC], bf16, tag="wv", name="wv")
            nc.sync.dma_start(
                out=wv, in_=wvT.rearrange("(c p) o -> p c o", p=P))
            wo = wp.tile([P, MCH, C], bf16, tag="wo", name="wo")
            nc.sync.dma_start(
                out=wo, in_=woT.rearrange("(m p) o -> p m o", p=P))

            def project_groups(tb, xc):
                groups = []

                def proj_qk(w, dst, m, tb=tb, xc=xc):
                    isq = dst is qt

                    def go():
                        ps = pp.tile([P, TB], f32, tag="mm512",
                                     name=f"ps_{tb}_{m}_{isq}")
                        for c in range(CCH):
                            nc.tensor.matmul(
                                ps, lhsT=w[m][:, c, :],
                                rhs=xsl(xc, c),
                                start=(c == 0), stop=(c == CCH - 1))
                        if isq:
                            for hh in (0, 1):
                                r0_ = hh * 64
                                nc.vector.tensor_copy(
                                    qt[2 * m + hh][r0_:r0_ + 64,
                                                   tb * TB:(tb + 1) * TB],
                                    ps[r0_:r0_ + 64, :])
                        else:
                            nc.vector.tensor_copy(
                                kt[m][:, tb * TB:(tb + 1) * TB], ps)
                    return go

                def proj_v(ts_, tb=tb, xc=xc):
                    def go():
                        ps = pp.tile([P, OC], f32, tag="mm512",
                                     name=f"psv_{tb}_{ts_}")
                        for c in range(CCH):
                            nc.tensor.matmul(
                                ps, lhsT=xsl(xc, c)[:, ts_ * P:(ts_ + 1) * P],
                                rhs=wv[:, c, :],
                                start=(c == 0), stop=(c == CCH - 1))
                        ti = tb * (TB // P) + ts_
                        nc.vector.tensor_copy(
                            v[ti].rearrange("p (h e) -> p h e", e=P)[:, :, 0:64],
                            ps.rearrange("p (h d) -> p h d", d=64))
                    return go

                for w, dst in ((wq, qt), (wk, kt)):
                    for m in range(MCH):
                        groups.append(proj_qk(w, dst, m))
                for ts_ in range(TB // P):
                    groups.append(proj_v(ts_))
                return groups

            def output_groups(tb):
                def out_co(co, tb=tb):
                    def go():
                        ps = pp.tile([P, TB], f32, tag="mm512",
                                     name=f"yps_{co}_{tb}")
                        for ci in range(MCH):
                            nc.tensor.matmul(
                                ps, lhsT=wo[:, ci, co * P:(co + 1) * P],
                                rhs=ct[ci][:, tb * TB:(tb + 1) * TB],
                                start=(ci == 0), stop=(ci == MCH - 1))
                        ysb = yp.tile([P, TB], f32, tag="ysb", name=f"ysb_{co}_{tb}")
                        nc.vector.tensor_copy(ysb, ps)
                        nc.sync.dma_start(
                            out=yT[co * P:(co + 1) * P, tb * TB:(tb + 1) * TB],
                            in_=ysb)
                    return go
                return [out_co(co) for co in range(C // P)]

            pending = []

            def mk_norm(h, j, m, r0, ctx_ps, fast=False):
                if fast:
                    # tail heads: nothing competes for the ctx PSUM bank any
                    # more — skip the cs staging copy to shorten the chain
                    # before the final output phase
                    def norm():
                        s_sb = tmp.tile([1, TB], f32, tag="r", name=f"s_{h}_{j}")
                        nc.vector.tensor_copy(s_sb, ctx_ps[64:65, :])
                        rb = tmp.tile([64, TB], f32, tag="rb", name=f"rb_{h}_{j}")
                        nc.gpsimd.partition_broadcast(rb, s_sb)
                        nc.vector.reciprocal_approx_fast(out=rb, in_=rb)
                        nc.vector.tensor_mul(
                            ct[m][r0:r0 + 64, j * TB:(j + 1) * TB],
                            ctx_ps[0:64, :], rb)
                    return norm
                if NORM_OLD:
                    def norm():
                        s_sb = tmp.tile([1, TB], f32, tag="r", name=f"s_{h}_{j}")
                        nc.vector.tensor_copy(s_sb, ctx_ps[64:65, :])
                        rb = tmp.tile([64, TB], f32, tag="rb", name=f"rb_{h}_{j}")
                        nc.gpsimd.partition_broadcast(rb, s_sb)
                        nc.vector.reciprocal_approx_fast(out=rb, in_=rb)
                        nc.vector.tensor_mul(
                            ct[m][r0:r0 + 64, j * TB:(j + 1) * TB],
                            ctx_ps[0:64, :], rb)
                    return norm

                # copy ctx+sums rows to SBUF first: frees the ctx PSUM bank
                # quickly instead of holding it through the whole recip chain.
                # reciprocal_approx_fast only reads base-partition-0 SBUF
                # reliably, so recip runs on the broadcast tile.
                def norm():
                    s_sb = tmp.tile([1, TB], f32, tag="r", name=f"s_{h}_{j}")
                    nc.vector.tensor_copy(s_sb, ctx_ps[64:65, :])
                    cs = tmp.tile([64, TB], f32, tag="cs", name=f"cs_{h}_{j}")
                    nc.vector.tensor_copy(cs, ctx_ps[0:64, :])
                    rb = tmp.tile([64, TB], f32, tag="rb", name=f"rb_{h}_{j}")
                    nc.gpsimd.partition_broadcast(rb, s_sb)
                    nc.vector.reciprocal_approx_fast(out=rb, in_=rb)
                    nc.vector.tensor_mul(
                        ct[m][r0:r0 + 64, j * TB:(j + 1) * TB], cs, rb)
                return norm

            def attend(j, ilq, late=()):
                late = list(late)
                reserve = ilq[-2:]
                main = ilq[:max(0, len(ilq) - 2)]
                # last block has no next-projection filler: spread its pops
                # thinner so PE filler survives into the exp-bound late heads
                npop = 2 if j == NTB - 1 else 3
                for h in range(HL):
                    if h >= 1:
                        for _ in range(npop):
                            if main:
                                main.pop(0)()
                    m, r0 = h // 2, (h % 2) * 64
                    nch = 4 * (j + 1)
                    qs = qt[h][:, j * TB:(j + 1) * TB]
                    ctx_ps = cxp.tile([P, TB], f32, tag="ctx", name=f"cps_{h}_{j}")
                    npair = nch // 2
                    # pair 0 first (full-width, carries the PSUM start), then
                    # the diagonal pairs so their long exp->mask chain overlaps
                    # later S matmuls, then the remaining non-diag pairs
                    if j == 0:
                        order = [0, 1]
                    else:
                        order = [0, npair - 1, npair - 2] + \
                            list(range(1, npair - 2))
                    inflight = []
                    nmm = [0]

                    def f0_of(c, j=j):
                        d = c - 4 * j
                        return d * P if d > 0 else 0  # fully-masked cols

                    def ctx_mms(pt_, pp0, ctx_ps=ctx_ps, h=h, nch=nch):
                        for t in (0, 1):
                            cc = 2 * pp0 + t
                            f0 = f0_of(cc)
                            nc.tensor.matmul(
                                ctx_ps[:, f0:],
                                lhsT=v[cc][:, h * P:(h + 1) * P],
                                rhs=pt_[:, t, f0:],
                                start=(nmm[0] == 0), stop=(nmm[0] == nch - 1),
                                skip_group_check=True)
                            nmm[0] += 1

                    for idx, pp_ in enumerate(order):
                        diag = 2 * pp_ >= 4 * j
                        mi = 0 if (j * TB - 2 * pp_ * P) == 0 else 1
                        # base-0 diag pair: one full-width ACT beats two
                        # sliced ones, so skip the S trim there (tiny cost)
                        strim = diag and mi == 1
                        st = stp.tile([P, 2 * TB], f32, tag="st",
                                      name=f"st_{h}_{j}_{pp_}")
                        for t in (0, 1):
                            c = 2 * pp_ + t
                            f0 = f0_of(c) if strim else 0
                            nc.tensor.matmul(
                                st[:, t * TB + f0:(t + 1) * TB],
                                lhsT=kt[m][:, c * P:(c + 1) * P],
                                rhs=qs[:, f0:], start=True, stop=True,
                                skip_group_check=True)
                        pt_ = ptp.tile([P, 2, TB], bf16, tag="pt",
                                       name=f"pt_{h}_{j}_{pp_}")
                        if strim:
                            # exp + triangle mask per chunk on exactly the
                            # causal-trimmed (written) column range
                            for t in (0, 1):
                                f0 = f0_of(2 * pp_ + t)
                                nc.scalar.activation(
                                    pt_[:, t, f0:],
                                    st[:, t * TB + f0:(t + 1) * TB], EXP,
                                    scale=SCALE / (WS * WS))
                                nc.vector.tensor_mul(
                                    pt_[:, t, f0:], pt_[:, t, f0:],
                                    masks[mi][:, t, f0:])
                        else:
                            nc.scalar.activation(
                                pt_.rearrange("p t f -> p (t f)"), st, EXP,
                                scale=SCALE / (WS * WS))
                            if diag:
                                nc.vector.tensor_mul(pt_, pt_, masks[mi])
                        if idx == 1 and pending:
                            pending.pop(0)()
                        if h == HL - 1 and idx >= 1:
                            # drip the last-block output phase A into head 7's
                            # pair loop: deps (ct[0..2]) are settled by now and
                            # the in-order PE queue never stalls on them
                            for _ in range(2):
                                if late:
                                    late.pop(0)()
                        inflight.append((pt_, pp_))
                        if len(inflight) > 2:
                            ctx_mms(*inflight.pop(0))
                    for it in inflight:
                        ctx_mms(*it)
                    fast = j == NTB - 1 and h >= HL - 2
                    pending.append(mk_norm(h, j, m, r0, ctx_ps, fast))
                for g in main + reserve:
                    g()
                for g in late:
                    g()
                # flush deferred norms so output(j) can run during project(j+1)
                while pending:
                    pending.pop(0)()

            # last block's output is split: ci 0-2 run inside attend(NTB-1)
            # once heads 0-5 are normed; ci 3 + combine run at the very end
            ysbA = {}

            def phaseA_groups():
                lo = (NTB - 1) * TB
                gs = []
                for co in range(C // P):
                    def go(co=co):
                        ps = pp.tile([P, TB], f32, tag="mm512",
                                     name=f"ypsA_{co}")
                        for ci in range(3):
                            nc.tensor.matmul(
                                ps, lhsT=wo[:, ci, co * P:(co + 1) * P],
                                rhs=ct[ci][:, lo:lo + TB],
                                start=(ci == 0), stop=(ci == 2))
                        ya = yp.tile([P, TB], f32, tag="ysbA", bufs=8,
                                     name=f"ysbA_{co}")
                        nc.vector.tensor_copy(ya, ps)
                        ysbA[co] = ya
                    gs.append(go)
                return gs

            def phaseB():
                lo = (NTB - 1) * TB
                for co in range(C // P):
                    ps = pp.tile([P, TB], f32, tag="mm512", name=f"ypsB_{co}")
                    nc.tensor.matmul(
                        ps, lhsT=wo[:, 3, co * P:(co + 1) * P],
                        rhs=ct[3][:, lo:lo + TB], start=True, stop=True)
                    ysb = yp.tile([P, TB], f32, tag="ysb", name=f"ysbB_{co}")
                    nc.vector.tensor_add(ysb, ps, ysbA[co])
                    nc.sync.dma_start(
                        out=yT[co * P:(co + 1) * P, lo:lo + TB], in_=ysb)

            x0 = x_next
            for g in project_groups(0, x_next):
                g()
            for tb in range(NTB):
                ilq = []
                if tb + 1 < NTB:
                    x_next = load_x(tb + 1)
                    ilq += project_groups(tb + 1, x_next)
                if tb >= 1:
                    ilq += output_groups(tb - 1)
                attend(tb, ilq,
                       late=phaseA_groups() if tb == NTB - 1 else ())
            phaseB()
            if DEBUG_DUMP:
                for nm, t_ in (("d_qt0", qt[0]), ("d_kt0", kt[0]),
                               ("d_v0", v[0]), ("d_ct0", ct[0]),
                               ("d_msk0", masks[0]), ("d_msk1", masks[1]),
                               ("d_wq", wq), ("d_x", x0)):
                    nc.sync.dma_start(out=dbg[nm], in_=t_)

    nc.compile()
    return nc


def make_in_maps(x, Wq, Wk, Wv, Wo):
    import ml_dtypes
    bf = ml_dtypes.bfloat16
    x = np.asarray(x, np.float32)
    Wq, Wk, Wv, Wo = (np.asarray(w, np.float32) for w in (Wq, Wk, Wv, Wo))
    in_maps = []
    for core in range(NCORES):
        b, g = divmod(core, GROUPS)
        sl = slice(g * OC, (g + 1) * OC)
        in_maps.append({
            "xT": np.ascontiguousarray(x[b].T).astype(bf),
            "wqT": np.ascontiguousarray((WS * Wq[sl, :]).T).astype(bf),
            "wkT": np.ascontiguousarray((WS * Wk[sl, :]).T).astype(bf),
            "wvT": np.ascontiguousarray(Wv[sl, :].T).astype(bf),
            "woT": np.ascontiguousarray(Wo[:, sl].T).astype(bf),
        })
    return in_maps


def _run(inputs, trace=False):
    from concourse.bass_utils import run_bass_kernel_spmd

    nc = build_program()
    in_maps = make_in_maps(
        inputs["x"], inputs["Wq"], inputs["Wk"], inputs["Wv"], inputs["Wo"])
    res = run_bass_kernel_spmd(nc, in_maps, core_ids=list(range(NCORES)), trace=trace)
    y = np.zeros((B, T_FULL, C), np.float32)
    for core in range(NCORES):
        y[core // GROUPS] += res.results[core]["yT"].T
    return y, res


def kernel(**inputs):
    y, _ = _run(inputs)
    return y
